# revision 1
# baseline (speedup 1.0000x reference)
"""Builder for the CausalWanModel sparse-attention TRN2 kernel.

Sharding (8 cores, 12 heads of HD=128):
  pair p in {0,1,2,3} owns heads {3p, 3p+1, 3p+2}; core 2p ("A") has
  slot0 = head 3p, core 2p+1 ("B") has slot0 = head 3p+2; both share
  slot1 = head 3p+1, split by attention window position: A covers
  cache[0:split] + all new tokens, B covers cache[split:L0]  (split =
  L1 - NEW so both see L1 "part1" keys; A's part1 is zero-padded and
  masked via exp bias).  The SPMD program is identical on all cores;
  only input data differs.

Collectives (all tiny, latency hidden behind compute):
  c1: AllReduce[all 8] of masked q sum-of-squares  (RMSNorm factors)
  c2: AllReduce[all 8] of masked k sum-of-squares
  c3: AllReduce[pairs] of slot1 softmax denominators

Attention uses transposed scores [s, t] so PV consumes exp tiles as the
moving operand directly (no P transpose); softmax skips max-subtraction
(|scores| <= sqrt(HD) after RMSNorm); denominators accumulate on DVE and
are partition-reduced with a ones-vector matmul.
"""

import math
import contextlib
import numpy as np

import concourse.bass as bass
import concourse.tile as tile
from concourse import bacc, mybir
from concourse.masks import make_identity

F32 = mybir.dt.float32
F32R = mybir.dt.float32r
AF = mybir.ActivationFunctionType
ALU = mybir.AluOpType

EPS = 1e-6
NEG_BIAS = -60.0  # exp(x + NEG_BIAS) ~ 0 for masked lanes


def subchunks(total, size=128):
    out = []
    off = 0
    while off < total:
        out.append((off, min(size, total - off)))
        off += size
    return out


def full_cfg():
    return dict(T=1560, NT=4, XD=1536, D=128, L0=7800, L1=4680, SUPER=512)


def build_program(cfg, n_cores=8):
    T, XD, D = cfg["T"], cfg["XD"], cfg["D"]
    NT = cfg["NT"]
    TC = T // NT
    assert TC * NT == T
    NK = XD // 128
    L0, L1, SUPER = cfg["L0"], cfg["L1"], cfg["SUPER"]
    NEW = T
    NJ = NK
    inv_sqrt_d = 1.0 / math.sqrt(D)
    new_subs = subchunks(NEW)
    n_new = len(new_subs)
    n_sub1 = len(subchunks(L1))

    nc = bacc.Bacc("TRN2", target_bir_lowering=False, debug=False,
                   num_devices=n_cores)

    def din(name, shape, dt=F32R):
        return nc.dram_tensor(name, shape, dt, kind="ExternalInput")

    xT_d = din("xT", [XD, T])
    w_d = {"q": din("wq", [XD, 256]), "k": din("wk", [XD, 256]),
           "v": din("wv", [XD, 256])}
    woT_d = din("woT", [256, XD])
    b_d = {"q": din("bq", [256, 1], F32), "k": din("bk", [256, 1], F32),
           "v": din("bv", [256, 1], F32)}
    g_d = {"q": din("gq", [256, 1], F32), "k": din("gk", [256, 1], F32)}
    bo_d = din("bo", [XD, 1], F32)
    cos2_d = din("cos2", [128, T], F32)
    sin2_d = din("sin2", [128, T], F32)
    sqmask_d = din("sqmask", [256, 1])
    kc0_d = din("kc0T", [128, L0])
    vc0_d = din("vc0", [L0, 128])
    kc1_d = din("kc1T", [128, L1])
    vc1_d = din("vc1", [L1, 128])
    ident_d = din("identc", [128, 128])
    swp_d = din("swpc", [128, 128])
    bias1_d = din("bias1", [128, n_sub1], F32)
    bias2_d = din("bias2", [128, n_new], F32)
    yT_d = nc.dram_tensor("yT", [XD, T], F32, kind="ExternalOutput")

    with tile.TileContext(nc) as tc, contextlib.ExitStack() as ctx:
        const = ctx.enter_context(tc.tile_pool(name="const", bufs=1))
        persist = ctx.enter_context(tc.tile_pool(name="persist", bufs=1))
        dram = ctx.enter_context(tc.tile_pool(name="dram", bufs=1, space="DRAM"))

        # ---- constants (host-built; memset/affine_select on f32r is
        # rejected by walrus ISA checks) ----
        ident = const.tile([128, 128], F32R)
        nc.sync.dma_start(ident[:], ident_d.ap())
        swp = const.tile([128, 128], F32R)
        nc.sync.dma_start(swp[:], swp_d.ap())
        ones_f32 = const.tile([128, 1], F32)
        nc.vector.memset(ones_f32[:], 1.0)
        eps_c = const.tile([1, 1], F32)
        nc.vector.memset(eps_c[:], EPS)

        # ---- small input tensors ----
        biases = const.tile([128, 6], F32)
        for i, nm in enumerate(("q", "k", "v")):
            nc.sync.dma_start(biases[:, 2 * i:2 * i + 1], b_d[nm].ap()[0:128, :])
            nc.sync.dma_start(biases[:, 2 * i + 1:2 * i + 2],
                              b_d[nm].ap()[128:256, :])
        gs = const.tile([128, 4], F32)
        for i, nm in enumerate(("q", "k")):
            nc.sync.dma_start(gs[:, 2 * i:2 * i + 1], g_d[nm].ap()[0:128, :])
            nc.sync.dma_start(gs[:, 2 * i + 1:2 * i + 2], g_d[nm].ap()[128:256, :])
        bo8 = const.tile([128, NJ], F32)
        bo_sb = const.tile([128, NJ], F32)
        nc.sync.dma_start(bo_sb[:].rearrange("p (j o) -> p j o", j=NJ),
                          bo_d.ap().rearrange("(j p) o -> p j o", p=128))
        nc.vector.tensor_scalar_mul(bo8[:], bo_sb[:], 1.0 / n_cores)
        sqmask = const.tile([128, 2], F32R)
        nc.sync.dma_start(sqmask[:, 0:1], sqmask_d.ap()[0:128, :])
        nc.sync.dma_start(sqmask[:, 1:2], sqmask_d.ap()[128:256, :])
        bias1 = const.tile([128, n_sub1], F32)
        nc.sync.dma_start(bias1[:], bias1_d.ap())
        bias2 = const.tile([128, n_new], F32)
        nc.sync.dma_start(bias2[:], bias2_d.ap())
        # collective bounce buffers
        cbuf = {}
        for nm in ("sqq", "sqk", "den1"):
            cin = dram.tile([1, T], F32, tag=f"cin_{nm}", name=f"cin_{nm}")
            cout = dram.tile([1, T], F32, tag=f"cout_{nm}", name=f"cout_{nm}")
            cbuf[nm] = (cin, cout)
        groups_all = [list(range(n_cores))]
        groups_pair = [[i, i + 1] for i in range(0, n_cores, 2)]

        # persistent across phases
        sq_sb = {nm: persist.tile([1, T], F32, tag=f"sq{nm}", name=f"sq{nm}") for nm in ("q", "k")}
        qkf = {nm: [persist.tile([128, T], F32R, tag=f"f{nm}{cc}", name=f"f{nm}{cc}")
                    for cc in range(2)] for nm in ("q", "k")}
        v_nat = [persist.tile([128, n_new * 128], F32R, tag=f"vn{cc}", name=f"vn{cc}")
                 for cc in range(2)]
        acc = [persist.tile([128, T], F32, tag=f"acc{s}", name=f"acc{s}") for s in range(2)]
        out1_sb = persist.tile([128, T], F32, tag="out1sb")
        den_sb = [persist.tile([1, T], F32, tag=f"den{s}", name=f"den{s}") for s in range(2)]

        # ================= P1/P2: projections + sumsq =================
        with tc.tile_pool(name="mid", bufs=1) as mid:
            xp = {nm: [mid.tile([128, T], F32R if nm == "v" else F32,
                                tag=f"x{nm}{cc}", name=f"x{nm}{cc}") for cc in range(2)]
                  for nm in ("q", "k", "v")}
            with tc.tile_pool(name="xT", bufs=1) as xpool, \
                 tc.tile_pool(name="wstr", bufs=4) as wpool, \
                 tc.tile_pool(name="proj_ps", bufs=1, space="PSUM") as pps, \
                 tc.tile_pool(name="sq_ps", bufs=1, space="PSUM") as sps, \
                 tc.tile_pool(name="sqt", bufs=3) as sqt_pool:

                xT_sb = xpool.tile([128, NK * T], F32R)
                nc.sync.dma_start(
                    xT_sb[:].rearrange("p (k t) -> p k t", k=NK),
                    xT_d.ap().rearrange("(k p) t -> p k t", p=128))

                for nm in ("q", "k", "v"):
                    for cc in range(2):
                        ps = [pps.tile([128, TC], F32, tag=f"proj{t}", name=f"proj{t}")
                              for t in range(NT)]
                        for kc in range(NK):
                            wt = wpool.tile([128, 128], F32R, tag="w")
                            nc.sync.dma_start(
                                wt[:], w_d[nm].ap()[kc * 128:(kc + 1) * 128,
                                                    cc * 128:(cc + 1) * 128])
                            for t in range(NT):
                                nc.tensor.matmul(
                                    ps[t][:], wt[:],
                                    xT_sb[:, kc * T + t * TC:kc * T + (t + 1) * TC],
                                    start=(kc == 0), stop=(kc == NK - 1))
                        ib = ("q", "k", "v").index(nm)
                        for t in range(NT):
                            nc.vector.tensor_scalar_add(
                                xp[nm][cc][:, t * TC:(t + 1) * TC], ps[t][:],
                                biases[:, 2 * ib + cc:2 * ib + cc + 1])
                    if nm in ("q", "k"):
                        for t in range(NT):
                            qps = sps.tile([1, TC], F32, tag="sqps")
                            for cc in range(2):
                                sqt = sqt_pool.tile([128, TC], F32R, tag="sqt")
                                nc.vector.tensor_tensor(
                                    out=sqt[:],
                                    in0=xp[nm][cc][:, t * TC:(t + 1) * TC],
                                    in1=xp[nm][cc][:, t * TC:(t + 1) * TC],
                                    op=ALU.mult)
                                nc.tensor.matmul(qps[:], sqmask[:, cc:cc + 1],
                                                 sqt[:], start=(cc == 0),
                                                 stop=(cc == 1))
                            nc.vector.tensor_copy(
                                sq_sb[nm][:, t * TC:(t + 1) * TC], qps[:])
                        key = "sq" + nm
                        nc.gpsimd.dma_start(cbuf[key][0][:], sq_sb[nm][:])
                        nc.gpsimd.collective_compute(
                            "AllReduce", ALU.add, replica_groups=groups_all,
                            ins=[cbuf[key][0].opt()], outs=[cbuf[key][1].opt()])

            # ================= P3/P4/P5: rms, rope, v-transpose ========
            with tc.tile_pool(name="rope", bufs=2) as rp, \
                 tc.tile_pool(name="ropec", bufs=1) as rpc, \
                 tc.tile_pool(name="rope_ps", bufs=3, space="PSUM") as rps:
                cos2 = rpc.tile([128, T], F32)
                sin2 = rpc.tile([128, T], F32)
                nc.sync.dma_start(cos2[:], cos2_d.ap())
                nc.sync.dma_start(sin2[:], sin2_d.ap())
                rms_b = {}
                for nm in ("q", "k"):
                    tot = rp.tile([1, T], F32, tag="rtmp", name="rtot")
                    nc.sync.dma_start(tot[:], cbuf["sq" + nm][1][:])
                    srt = rp.tile([1, T], F32, tag="rtmp", name="rsrt")
                    nc.scalar.activation(srt[:], tot[:], AF.Sqrt,
                                         bias=eps_c[:], scale=1.0 / XD)
                    rec = rp.tile([1, T], F32, tag="rtmp", name="rrec")
                    nc.vector.reciprocal(rec[:], srt[:])
                    if nm == "q":
                        rec2 = rp.tile([1, T], F32, tag="rtmp", name="rrec2")
                        nc.vector.tensor_scalar_mul(rec2[:], rec[:], inv_sqrt_d)
                        rec = rec2
                    rms_b[nm] = rp.tile([128, T], F32, tag=f"rms{nm}", name=f"rms{nm}", bufs=1)
                    nc.gpsimd.partition_broadcast(rms_b[nm][:], rec[0:1, :])

                for ig, nm in enumerate(("q", "k")):
                    for cc in range(2):
                        xg = rp.tile([128, T], F32R, tag="xg")
                        nc.vector.tensor_scalar_mul(
                            xg[:], xp[nm][cc][:],
                            gs[:, 2 * ig + cc:2 * ig + cc + 1])
                        m1 = rp.tile([128, T], F32, tag="m1")
                        nc.vector.tensor_tensor(out=m1[:], in0=xg[:],
                                                in1=cos2[:], op=ALU.mult)
                        rq = rp.tile([128, T], F32, tag="rq")
                        for t in range(NT):
                            swps = rps.tile([128, TC], F32, tag="swp")
                            nc.tensor.matmul(swps[:], swp[:],
                                             xg[:, t * TC:(t + 1) * TC],
                                             start=True, stop=True)
                            m2 = rp.tile([128, TC], F32, tag="m2")
                            nc.vector.tensor_tensor(
                                out=m2[:], in0=swps[:],
                                in1=sin2[:, t * TC:(t + 1) * TC], op=ALU.mult)
                            nc.vector.tensor_tensor(
                                out=rq[:, t * TC:(t + 1) * TC],
                                in0=m1[:, t * TC:(t + 1) * TC], in1=m2[:],
                                op=ALU.add)
                        nc.vector.tensor_tensor(out=qkf[nm][cc][:], in0=rq[:],
                                                in1=rms_b[nm][:], op=ALU.mult)

                for cc in range(2):
                    for j, (off, ck) in enumerate(new_subs):
                        tp = rps.tile([128, 128], F32R, tag="vtp")
                        nc.tensor.matmul(tp[0:ck, :],
                                         xp["v"][cc][:, off:off + ck], ident[:],
                                         is_transpose=True, start=True, stop=True)
                        nc.vector.tensor_copy(
                            v_nat[cc][0:ck, j * 128:(j + 1) * 128], tp[0:ck, :])

        # ================= P6/P7: attention =================
        with tc.tile_pool(name="outps", bufs=1, space="PSUM") as ops, \
             tc.tile_pool(name="dps", bufs=1, space="PSUM") as dpool:

            def attn_chunk(slot, k_ap, v_ap, ck, btile, bidx, first, last,
                           out_ps, ep_, stp):
                for t in range(NT):
                    st = stp.tile([128, TC], F32, tag="st")
                    nc.tensor.matmul(st[0:ck, :], k_ap,
                                     qkf["q"][slot][:, t * TC:(t + 1) * TC],
                                     start=True, stop=True)
                    ex = ep_.tile([128, TC], F32R, tag="ex")
                    bias = 0.0 if btile is None else btile[0:ck, bidx:bidx + 1]
                    nc.scalar.activation(ex[0:ck, :], st[0:ck, :], AF.Exp,
                                         bias=bias)
                    if first:
                        assert ck == 128
                        nc.vector.tensor_copy(acc[slot][:, t * TC:(t + 1) * TC],
                                              ex[:, :])
                    else:
                        nc.vector.tensor_add(
                            acc[slot][0:ck, t * TC:(t + 1) * TC],
                            acc[slot][0:ck, t * TC:(t + 1) * TC], ex[0:ck, :])
                    nc.tensor.matmul(out_ps[t][:], v_ap, ex[0:ck, :],
                                     start=first, stop=last,
                                     skip_group_check=True)

            def attend(slot, segments, out_ps, evac_to):
                gidx = 0
                total = sum(len(subchunks(L)) for _, _, _, L, _ in segments)
                with tc.tile_pool(name=f"att{slot}", bufs=3) as ap_, \
                     tc.tile_pool(name=f"exp{slot}", bufs=4) as ep_, \
                     tc.tile_pool(name=f"st{slot}", bufs=2, space="PSUM") as stp:
                    for kind, ksrc, vsrc, L, btile in segments:
                        if kind == "dram":
                            for soff, ssz in subchunks(L, SUPER):
                                ks = ap_.tile([128, SUPER], F32R, tag="ks")
                                nc.sync.dma_start(ks[:, 0:ssz],
                                                  ksrc.ap()[:, soff:soff + ssz])
                                vs = ap_.tile([128, SUPER], F32R, tag="vs")
                                subs = subchunks(ssz)
                                for j, (o2, c2) in enumerate(subs):
                                    nc.sync.dma_start(
                                        vs[0:c2, j * 128:(j + 1) * 128],
                                        vsrc.ap()[soff + o2:soff + o2 + c2, :])
                                for j, (o2, c2) in enumerate(subs):
                                    attn_chunk(slot, ks[:, o2:o2 + c2],
                                               vs[0:c2, j * 128:(j + 1) * 128],
                                               c2, btile, (soff + o2) // 128,
                                               gidx == 0, gidx == total - 1,
                                               out_ps, ep_, stp)
                                    gidx += 1
                        else:
                            for j, (off, ck) in enumerate(subchunks(L)):
                                attn_chunk(slot, ksrc[:, off:off + ck],
                                           v_nat[slot][0:ck, j * 128:(j + 1) * 128],
                                           ck, btile, j,
                                           gidx == 0, gidx == total - 1,
                                           out_ps, ep_, stp)
                                gidx += 1
                for t in range(NT):
                    dps = dpool.tile([1, TC], F32, tag="dps")
                    nc.tensor.matmul(dps[:], ones_f32[:],
                                     acc[slot][:, t * TC:(t + 1) * TC],
                                     start=True, stop=True)
                    nc.vector.tensor_copy(den_sb[slot][:, t * TC:(t + 1) * TC],
                                          dps[:])
                if evac_to is not None:
                    for t in range(NT):
                        nc.scalar.copy(evac_to[:, t * TC:(t + 1) * TC],
                                       out_ps[t][:])

            out1_ps = [ops.tile([128, TC], F32, tag=f"o_{t}", name=f"o1_{t}") for t in range(NT)]
            attend(1, [("dram", kc1_d, vc1_d, L1, bias1),
                       ("sbuf", qkf["k"][1], None, NEW, bias2)],
                   out1_ps, out1_sb)
            nc.gpsimd.dma_start(cbuf["den1"][0][:], den_sb[1][:])
            nc.gpsimd.collective_compute(
                "AllReduce", ALU.add, replica_groups=groups_pair,
                ins=[cbuf["den1"][0].opt()], outs=[cbuf["den1"][1].opt()])

            out0_ps = [ops.tile([128, TC], F32, tag=f"o_{t}", name=f"o0_{t}") for t in range(NT)]
            attend(0, [("dram", kc0_d, vc0_d, L0, None),
                       ("sbuf", qkf["k"][0], None, NEW, None)],
                   out0_ps, None)

            # ================= P8: normalize + out projection ==========
            with tc.tile_pool(name="fin", bufs=2) as fp_, \
                 tc.tile_pool(name="yps", bufs=3, space="PSUM") as yps:
                woT_sb = fp_.tile([128, 2 * XD], F32R, bufs=1)
                nc.sync.dma_start(woT_sb[:, 0:XD], woT_d.ap()[0:128, :])
                nc.sync.dma_start(woT_sb[:, XD:2 * XD], woT_d.ap()[128:256, :])
                den1_tot = fp_.tile([1, T], F32, tag="d1t", bufs=1)
                nc.sync.dma_start(den1_tot[:], cbuf["den1"][1][:])
                recips = []
                for slot, dsrc in ((0, den_sb[0]), (1, den1_tot)):
                    rc = fp_.tile([1, T], F32, tag=f"rc{slot}", name=f"rc{slot}", bufs=1)
                    nc.vector.reciprocal(rc[:], dsrc[:])
                    rb = fp_.tile([128, T], F32, tag=f"rb{slot}", name=f"rb{slot}", bufs=1)
                    nc.gpsimd.partition_broadcast(rb[:], rc[0:1, :])
                    recips.append(rb)
                of = []
                for slot in (0, 1):
                    o = fp_.tile([128, T], F32R, tag=f"of{slot}", name=f"of{slot}", bufs=1)
                    for t in range(NT):
                        in0 = out0_ps[t][:] if slot == 0 else \
                            out1_sb[:, t * TC:(t + 1) * TC]
                        nc.vector.tensor_tensor(
                            out=o[:, t * TC:(t + 1) * TC], in0=in0,
                            in1=recips[slot][:, t * TC:(t + 1) * TC],
                            op=ALU.mult)
                    of.append(o)
                for jc in range(NJ):
                    for t in range(NT):
                        yp = yps.tile([128, TC], F32, tag="yp")
                        nc.tensor.matmul(
                            yp[:], woT_sb[:, jc * 128:(jc + 1) * 128],
                            of[0][:, t * TC:(t + 1) * TC],
                            start=True, stop=False)
                        nc.tensor.matmul(
                            yp[:], woT_sb[:, XD + jc * 128:XD + (jc + 1) * 128],
                            of[1][:, t * TC:(t + 1) * TC],
                            start=False, stop=True)
                        ysb = fp_.tile([128, TC], F32, tag="ysb")
                        nc.scalar.activation(ysb[:], yp[:], AF.Identity,
                                             bias=bo8[:, jc:jc + 1])
                        nc.sync.dma_start(
                            yT_d.ap()[jc * 128:(jc + 1) * 128,
                                      t * TC:(t + 1) * TC], ysb[:])

    nc.compile()
    return nc


# ---------------- host side ----------------

def host_prepare(cfg, x, freqs_cos, freqs_sin, wq, bq, wk, bk, wv, bv,
                 wo, bo, gq, gk, win_old_k, win_old_v, n_cores=8):
    """win_old_k/v: [L0, XD] assembled old window (eviction applied)."""
    T, XD = cfg["T"], cfg["XD"]
    L0, L1 = cfg["L0"], cfg["L1"]
    NEW = T
    split = L1 - NEW
    assert 0 <= split <= L0 and L0 - split <= L1, (L0, L1, split)
    n_sub1 = len(subchunks(L1))
    n_new = len(subchunks(NEW))

    f32 = np.float32
    xT = np.ascontiguousarray(x.reshape(T, XD).T, f32)
    cos2 = np.ascontiguousarray(np.concatenate([freqs_cos.T, freqs_cos.T], 0), f32)
    sin2 = np.ascontiguousarray(np.concatenate([freqs_sin.T, freqs_sin.T], 0), f32)

    in_maps = []
    for c in range(n_cores):
        p, role = divmod(c, 2)
        h0 = 3 * p + (0 if role == 0 else 2)
        h1 = 3 * p + 1
        cols = np.r_[h0 * 128:(h0 + 1) * 128, h1 * 128:(h1 + 1) * 128]
        cols0 = np.r_[h0 * 128:(h0 + 1) * 128]
        cols1 = np.r_[h1 * 128:(h1 + 1) * 128]

        m = {"xT": xT, "cos2": cos2, "sin2": sin2}
        m["wq"] = np.ascontiguousarray(wq[cols, :].T, f32)
        m["wk"] = np.ascontiguousarray(wk[cols, :].T, f32)
        m["wv"] = np.ascontiguousarray(wv[cols, :].T, f32)
        m["woT"] = np.ascontiguousarray(wo[:, cols].T, f32)
        m["bq"] = np.ascontiguousarray(bq[cols][:, None], f32)
        m["bk"] = np.ascontiguousarray(bk[cols][:, None], f32)
        m["bv"] = np.ascontiguousarray(bv[cols][:, None], f32)
        m["gq"] = np.ascontiguousarray(gq[cols][:, None], f32)
        m["gk"] = np.ascontiguousarray(gk[cols][:, None], f32)
        m["bo"] = np.ascontiguousarray(bo[:, None], f32)

        sqmask = np.zeros((256, 1), f32)
        sqmask[0:128] = 1.0
        q4 = 32
        if role == 0:
            sqmask[128:128 + q4] = 1.0
            sqmask[128 + 2 * q4:128 + 3 * q4] = 1.0
        else:
            sqmask[128 + q4:128 + 2 * q4] = 1.0
            sqmask[128 + 3 * q4:] = 1.0
        m["sqmask"] = sqmask
        m["identc"] = np.eye(128, dtype=f32)
        swpc = np.zeros((128, 128), f32)
        swpc[np.arange(64), np.arange(64) + 64] = 1.0
        swpc[np.arange(64) + 64, np.arange(64)] = -1.0
        m["swpc"] = swpc

        m["kc0T"] = np.ascontiguousarray(win_old_k[:, cols0].T, f32)
        m["vc0"] = np.ascontiguousarray(win_old_v[:, cols0], f32)

        k1 = np.zeros((L1, 128), f32)
        v1 = np.zeros((L1, 128), f32)
        if role == 0:
            k1[0:split] = win_old_k[0:split][:, cols1]
            v1[0:split] = win_old_v[0:split][:, cols1]
            valid1 = split
        else:
            valid1 = L0 - split
            k1[0:valid1] = win_old_k[split:L0][:, cols1]
            v1[0:valid1] = win_old_v[split:L0][:, cols1]
        m["kc1T"] = np.ascontiguousarray(k1.T)
        m["vc1"] = v1

        bias1 = np.zeros((128, n_sub1), f32)
        for j, (off, ck) in enumerate(subchunks(L1)):
            lv = int(np.clip(valid1 - off, 0, 128))
            bias1[lv:, j] = NEG_BIAS
        m["bias1"] = bias1
        bias2 = np.zeros((128, n_new), f32)
        if role == 1:
            bias2[:] = NEG_BIAS
        m["bias2"] = bias2
        in_maps.append(m)
    return in_maps


def host_finalize(cfg, results):
    y = None
    for r in results:
        y = r["yT"].astype(np.float64) if y is None \
            else y + r["yT"].astype(np.float64)
    return np.ascontiguousarray(y.T)[None].astype(np.float32)  # [1, T, XD]


def numpy_reference(cfg, x, freqs_cos, freqs_sin, wq, bq, wk, bk, wv, bv,
                    wo, bo, gq, gk, win_old_k, win_old_v):
    """Reference for arbitrary cfg: attention over [old window; new]."""
    T, XD, D = cfg["T"], cfg["XD"], cfg["D"]
    H = XD // D
    x2 = x.reshape(T, XD).astype(np.float64)

    def rms(t, g):
        return t / np.sqrt((t ** 2).mean(-1, keepdims=True) + EPS) * g

    q = rms(x2 @ wq.T + bq, gq)
    k = rms(x2 @ wk.T + bk, gk)
    v = x2 @ wv.T + bv

    def rope(t):
        th = t.reshape(T, H, D)
        t1, t2 = th[..., :D // 2], th[..., D // 2:]
        c = freqs_cos[:, None, :]
        s = freqs_sin[:, None, :]
        return np.concatenate([t1 * c - t2 * s, t1 * s + t2 * c],
                              -1).reshape(T, XD)

    rq, rk = rope(q), rope(k)
    kw = np.concatenate([win_old_k, rk], 0).reshape(-1, H, D)
    vw = np.concatenate([win_old_v, v], 0).reshape(-1, H, D)
    qh = rq.reshape(T, H, D)
    scores = np.einsum("thd,shd->hts", qh, kw) / math.sqrt(D)
    e = np.exp(scores - scores.max(-1, keepdims=True))
    probs = e / e.sum(-1, keepdims=True)
    out = np.einsum("hts,shd->thd", probs, vw).reshape(T, XD)
    return (out @ wo.T + bo)[None].astype(np.float32)


# =====================================================================
# kernel() entry point — full inputs in, full output out.
# =====================================================================

import os as _os
import time as _time
from concourse import bass_utils as _bass_utils

_DIM = 1536
_HEADS = 12
_HD = 128
_FRAME = 1560
_LOCAL_ATTN_SIZE = 6
_SINK_SIZE = 1
_CACHE = _LOCAL_ATTN_SIZE * _FRAME
_SINK = _SINK_SIZE * _FRAME
_MAX_ATTN = _LOCAL_ATTN_SIZE * _FRAME
_GLOBAL_END = _CACHE
_LOCAL_END = _CACHE

_prog_cache = {}
last_exec_ns = None
last_wall_ns = None


def _window_index(current_start, T):
    """Mirrors the reference's rolling-cache index math; returns original
    cache row indices of the attention window's old part."""
    cur_end = current_start + T
    if cur_end > _GLOBAL_END and T + _LOCAL_END > _CACHE:
        evict = T + _LOCAL_END - _CACHE
        rolled = _LOCAL_END - evict - _SINK
        le = _LOCAL_END + cur_end - _GLOBAL_END - evict
    else:
        evict, rolled = 0, 0
        le = _LOCAL_END + cur_end - _GLOBAL_END
    ls = le - T
    ws = max(0, le - _MAX_ATTN)
    idx = np.arange(ws, ls)
    if evict:
        shift = (idx >= _SINK) & (idx < _SINK + rolled)
        idx = np.where(shift, idx + evict, idx)
    return idx


def kernel(**inputs):
    global last_exec_ns, last_wall_ns
    x = np.ascontiguousarray(np.asarray(inputs["x"], np.float32))
    B, T, XD = x.shape
    assert B == 1 and XD == _DIM
    cs = int(np.asarray(inputs["current_start"]))
    idx_old = _window_index(cs, T)
    L0 = len(idx_old)
    L1 = max(T, (L0 + T + 1) // 2)
    NT = 1
    for cand in (4, 3, 2):
        if T % cand == 0 and T // cand <= 512:
            NT = cand
            break
    if T <= 512:
        NT = 1
    cfg = dict(T=T, NT=NT, XD=XD, D=_HD, L0=L0, L1=L1, SUPER=512)

    key = tuple(sorted(cfg.items()))
    if key not in _prog_cache:
        _prog_cache[key] = build_program(cfg, n_cores=8)
    nc = _prog_cache[key]

    f32 = np.float32
    ck = np.asarray(inputs["cache_k"], f32)[0]
    cv = np.asarray(inputs["cache_v"], f32)[0]
    win_k = np.ascontiguousarray(ck[idx_old])
    win_v = np.ascontiguousarray(cv[idx_old])
    args = (x, np.asarray(inputs["freqs_cos"], f32),
            np.asarray(inputs["freqs_sin"], f32),
            np.asarray(inputs["wq"], f32), np.asarray(inputs["bq"], f32),
            np.asarray(inputs["wk"], f32), np.asarray(inputs["bk"], f32),
            np.asarray(inputs["wv"], f32), np.asarray(inputs["bv"], f32),
            np.asarray(inputs["wo"], f32), np.asarray(inputs["bo"], f32),
            np.asarray(inputs["gq"], f32), np.asarray(inputs["gk"], f32),
            win_k, win_v)
    in_maps = host_prepare(cfg, *args, n_cores=8)

    trace = bool(int(_os.environ.get("WAN_KERNEL_TRACE", "0")))
    t0 = _time.time()
    try:
        res = _bass_utils.run_bass_kernel_spmd(
            nc, in_maps, core_ids=list(range(8)), trace=trace)
    except (ImportError, ModuleNotFoundError):
        # NTFF profile hook unavailable on this client image
        res = _bass_utils.run_bass_kernel_spmd(
            nc, in_maps, core_ids=list(range(8)))
    last_wall_ns = int((_time.time() - t0) * 1e9)
    last_exec_ns = res.exec_time_ns
    return host_finalize(cfg, [res.results[c] for c in range(8)])



# revision 26
# speedup vs baseline: 2.0154x; 2.0154x over previous
"""Builder for the CausalWanModel sparse-attention TRN2 kernel (v4).

Sharding (8 cores, 12 heads of HD=128):
  pair p in {0,1,2,3} owns heads {3p, 3p+1, 3p+2}; core 2p ("A") has
  slot0 = head 3p, core 2p+1 ("B") has slot0 = head 3p+2; both share
  slot1 = head 3p+1, split by attention window position: A covers
  cache[0:split] + all new tokens, B covers cache[split:L0]  (split =
  L1 - NEW so both see L1 "part1" keys; A's part1 is zero-padded and
  masked via exp bias).  The SPMD program is identical on all cores;
  only input data differs.

v4 structure:
  - the Activation engine is the intrinsic bottleneck (~510ns per exp
    of a [128,390] score tile, no dtype speedup); everything else is
    arranged so Act never waits;
  - attention runs in two query-half passes so the score PSUM ring is
    4 deep (2 out banks + 1 den bank + 4 st banks): the score matmul
    leads its exp by several units without PSUM WAR stalls;
  - new-key RMSNorm folds into the exp's per-partition scale (lamk):
    the k-sumsq collective is fully off the critical path;
  - q-norm uses AllGather (15us model cost) + an on-device ones-matmul
    row sum instead of AllReduce (28us);
  - q projection streams kc-outer into 8 PSUM banks so both column
    halves finish with the last x chunk; sum-of-squares runs on the
    idle Act engine (Square);
  - softmax denominators: bf16 exp tiles pair-summed on DVE, then a
    ones-stationary matmul per pair accumulates into one PSUM bank;
  - x, weights, V (cache + new) and the output stream in bf16; scores
    and Q/K stay f32/f32r; V transposes use the DMA XBAR (one
    instruction per column half);
  - small constants ship as one packed [128,74] tensor (HWDGE issue
    slots cost 625ns each).

Collectives: c1 AllGather[8] q-sumsq; c2 AllReduce[8] k-sumsq;
c3 AllReduce[pairs] slot1 denominators.
"""

import math
import contextlib
import numpy as np

import concourse.bass as bass
import concourse.tile as tile
from concourse import bacc, mybir

F32 = mybir.dt.float32
F32R = mybir.dt.float32r
BF16 = mybir.dt.bfloat16
AF = mybir.ActivationFunctionType
ALU = mybir.AluOpType

EPS = 1e-6
NEG_BIAS = -60.0  # exp(x + NEG_BIAS) ~ 0 for masked lanes


def subchunks(total, size=128):
    out = []
    off = 0
    while off < total:
        out.append((off, min(size, total - off)))
        off += size
    return out


def full_cfg():
    return dict(T=1560, NT=4, XD=1536, D=128, L0=7800, L1=4680, SUPER=512)


def build_program(cfg, n_cores=8):
    T, XD, D = cfg["T"], cfg["XD"], cfg["D"]
    NT = cfg["NT"]
    TC = T // NT
    assert TC * NT == T
    NPASS = 2 if NT % 2 == 0 and NT > 1 else 1
    TP = NT // NPASS
    NK = XD // 128
    L0, L1, SUPER = cfg["L0"], cfg["L1"], cfg["SUPER"]
    NEW = T
    NJ = NK
    new_subs = subchunks(NEW)
    n_new = len(new_subs)
    n_sub1 = len(subchunks(L1))
    TFULL = (T // 128) * 128
    TREM = T - TFULL
    NS = 6 + 4 + NJ + 2 + n_sub1 + n_new

    nc = bacc.Bacc("TRN2", target_bir_lowering=False, debug=False,
                   num_devices=n_cores)

    def din(name, shape, dt=F32R):
        return nc.dram_tensor(name, shape, dt, kind="ExternalInput")

    xT_d = din("xT", [XD, T], BF16)
    w_d = {"q": din("wq", [128, NK * 256], BF16),
           "k": din("wk", [128, NK * 256], BF16),
           "v": din("wv", [128, NK * 256], BF16)}
    woT_d = din("woT", [256, XD], BF16)
    smalls_d = din("smalls", [128, NS], F32)
    swp_d = din("swpc", [128, 128], BF16)
    cossin_d = din("cossin", [128, 2 * T], BF16)
    kc0_d = din("kc0T", [128, L0], BF16)
    vc0_d = din("vc0", [L0, 128], BF16)
    kc1_d = din("kc1T", [128, L1], BF16)
    vc1_d = din("vc1", [L1, 128], BF16)
    yT_d = nc.dram_tensor("yT", [XD, T], BF16, kind="ExternalOutput")

    with tile.TileContext(nc) as tc, contextlib.ExitStack() as ctx:
        const = ctx.enter_context(tc.tile_pool(name="const", bufs=1))
        persist = ctx.enter_context(tc.tile_pool(name="persist", bufs=1))
        pf = ctx.enter_context(tc.tile_pool(name="pf", bufs=1))
        dram = ctx.enter_context(tc.tile_pool(name="dram", bufs=1, space="DRAM"))

        # ---- constants (no DMAs here; issued in startup-critical order) ----
        ones_f32 = const.tile([128, 1], F32)
        nc.vector.memset(ones_f32[:], 1.0)
        ones_r = const.tile([128, 1], F32R)
        nc.vector.tensor_copy(ones_r[:], ones_f32[:])
        ones_bf = const.tile([128, 1], BF16)
        nc.vector.tensor_copy(ones_bf[:], ones_f32[:])
        onesrow_f = const.tile([1, 128], F32)
        nc.vector.memset(onesrow_f[:], 1.0)
        onesrow = const.tile([1, 128], F32R)
        nc.vector.tensor_copy(onesrow[:], onesrow_f[:])
        eps_cD = const.tile([1, 1], F32)
        nc.vector.memset(eps_cD[:], EPS * D)
        eps_col = const.tile([128, 1], F32)
        nc.vector.memset(eps_col[:], EPS)

        smalls = const.tile([128, NS], F32)
        swp = const.tile([128, 128], BF16)
        cossin = const.tile([128, 2 * T], BF16)
        cos2 = cossin[:, 0:T]
        sin2 = cossin[:, T:2 * T]
        sqmask = const.tile([128, 2], F32R)
        biases = smalls[:, 0:6]
        gs = smalls[:, 6:10]
        bo8 = smalls[:, 10:10 + NJ]           # pre-scaled by 1/n_cores on host
        sqmask_f = smalls[:, 10 + NJ:12 + NJ]
        bias1 = smalls[:, 12 + NJ:12 + NJ + n_sub1]
        bias2 = smalls[:, 12 + NJ + n_sub1:NS]

        # collective bounce buffers
        cin_q = dram.tile([1, T], F32, tag="cin_q", name="cin_q")
        cout_q = dram.tile([8, T], F32, tag="cout_q", name="cout_q")
        TPAD = n_new * 128
        cin_k = dram.tile([1, TPAD], F32, tag="cin_k", name="cin_k")
        cout_k = dram.tile([1, TPAD], F32, tag="cout_k", name="cout_k")
        cin_d = dram.tile([1, T], F32, tag="cin_d", name="cin_d")
        cout_d = dram.tile([2, T], F32, tag="cout_d", name="cout_d")
        groups_all = [list(range(n_cores))]
        groups_pair = [[i, i + 1] for i in range(0, n_cores, 2)]

        # persistent across phases (incl. inputs consumed by background-
        # dripped work during attention: x chunks, wv, xp_v)
        qkf = {nm: [persist.tile([128, T], BF16, tag=f"f{nm}{cc}", name=f"f{nm}{cc}")
                    for cc in range(2)] for nm in ("q", "k")}
        v_nat = [persist.tile([128, n_new * 128], BF16, tag=f"vn{cc}", name=f"vn{cc}")
                 for cc in range(2)]
        lamk = persist.tile([128, n_new], F32, tag="lamk")
        lamk_sq = persist.tile([128, n_new], F32, tag="lamksq")
        lamk_srt = persist.tile([128, n_new], F32, tag="lamksrt")
        rec2 = persist.tile([1, T], F32R, tag="rec2")
        xp_v = [persist.tile([128, n_new * 128], BF16, tag=f"xv{cc}",
                             name=f"xv{cc}") for cc in range(2)]
        xp_k = [persist.tile([128, T], BF16, tag=f"xkk{cc}",
                             name=f"xkk{cc}") for cc in range(2)]
        xk = [persist.tile([128, T], BF16, tag=f"xk{kc}", name=f"xk{kc}")
              for kc in range(NK)]
        wv_t = persist.tile([128, NK * 256], BF16, tag="wv")

        def vs_dma(vs_tile, vsrc, soff, ssz):
            nj = (ssz + 127) // 128
            if ssz % 128 == 0:
                nc.sync.dma_start(
                    vs_tile[:, 0:ssz].rearrange("p (j d) -> p j d", j=nj),
                    vsrc.ap()[soff:soff + ssz, :]
                        .rearrange("(j p) d -> p j d", p=128))
            else:
                for j, (o2, c2) in enumerate(subchunks(ssz)):
                    nc.sync.dma_start(
                        vs_tile[0:c2, j * 128:(j + 1) * 128],
                        vsrc.ap()[soff + o2:soff + o2 + c2, :])

        pre = {}
        rope_steps_fn = [None]

        # ================= P1-P3: projections, norms, rope =================
        with tc.tile_pool(name="mid", bufs=1) as mid:
            xp = {}
            xp["q"] = [mid.tile([128, T], BF16, tag=f"xq{cc}",
                                name=f"xq{cc}") for cc in range(2)]
            xp["k"] = xp_k
            xp["v"] = xp_v
            sq_sb = {nm: mid.tile([1, T], F32, tag=f"sq{nm}", name=f"sq{nm}")
                     for nm in ("q", "k")}
            dummy = mid.tile([128, TC], F32, tag="dummy")
            with tc.tile_pool(name="wstr", bufs=2) as wpool, \
                 tc.tile_pool(name="rope", bufs=2) as rp, \
                 tc.tile_pool(name="sqt", bufs=3) as sqt_pool:

                wts = {}

                def wdma(nm, pool_tile=None):
                    wt = pool_tile if pool_tile is not None else \
                        wpool.tile([128, NK * 256], BF16, tag="w", name=f"w{nm}")
                    nc.sync.dma_start(wt[:], w_d[nm].ap())
                    wts[nm] = wt

                # startup-critical DMA issue order
                wdma("q")
                if TPAD > T:
                    zpad = mid.tile([1, TPAD - T], F32, tag="zpad")
                    nc.vector.memset(zpad[:], 0.0)
                    nc.gpsimd.dma_start(cin_k[0:1, T:TPAD], zpad[:])
                for kc in range(4):
                    nc.sync.dma_start(xk[kc][:],
                                      xT_d.ap()[kc * 128:(kc + 1) * 128, :])
                nc.sync.dma_start(smalls[:], smalls_d.ap())
                nc.vector.tensor_copy(sqmask[:], sqmask_f)
                for kc in range(4, NK):
                    nc.sync.dma_start(xk[kc][:],
                                      xT_d.ap()[kc * 128:(kc + 1) * 128, :])
                nc.sync.dma_start(swp[:], swp_d.ap())
                wdma("k")
                nc.sync.dma_start(cossin[:], cossin_d.ap())
                # prefetch super-0 K/V for both attention slots
                for slot, (kd, vd, L) in ((1, (kc1_d, vc1_d, L1)),
                                          (0, (kc0_d, vc0_d, L0))):
                    ssz = min(SUPER, L)
                    pks = pf.tile([128, SUPER], BF16, tag=f"pks{slot}",
                                  name=f"pks{slot}")
                    nc.sync.dma_start(pks[:, 0:ssz], kd.ap()[:, 0:ssz])
                    pvs = pf.tile([128, SUPER], BF16, tag=f"pvs{slot}",
                                  name=f"pvs{slot}")
                    vs_dma(pvs, vd, 0, ssz)
                    pre[slot] = {0: (pks, pvs)}
                wdma("v", wv_t)

                def bias_evac(nm, cc, ps_list):
                    ib = ("q", "k", "v").index(nm)
                    dst = xp[nm][cc]
                    for t in range(NT):
                        nc.vector.tensor_scalar_add(
                            dst[:, t * TC:(t + 1) * TC], ps_list[t][:],
                            biases[:, 2 * ib + cc:2 * ib + cc + 1])

                def sumsq(nm, sps):
                    # squares on the (idle) Act engine from SBUF
                    for t in range(NT):  # noqa
                        qps = sps.tile([1, TC], F32, tag="sqps")
                        for cc in range(2):
                            sqt = sqt_pool.tile([128, TC], F32R, tag="sqt")
                            nc.scalar.activation(
                                sqt[:], xp[nm][cc][:, t * TC:(t + 1) * TC],
                                AF.Square)
                            nc.tensor.matmul(qps[:], sqmask[:, cc:cc + 1],
                                             sqt[:], start=(cc == 0),
                                             stop=(cc == 1))
                        nc.vector.tensor_copy(
                            sq_sb[nm][:, t * TC:(t + 1) * TC], qps[:])

                def rope_steps(nm, cc, pool, psum_pool, psum_tag):
                    """Per-t steps: qkf = (xp*g)*cos + swp@(xp*g)*sin.
                    Returns a list of closures (bg-drippable)."""
                    ig = ("q", "k").index(nm)
                    out_tile = qkf[nm][cc]
                    steps = []
                    for t in range(NT):
                        cell = {}

                        def s1(t=t, cell=cell):
                            lo = t * TC
                            xg = pool.tile([128, TC], BF16, tag="rxg",
                                           name=f"rxg")
                            nc.vector.tensor_scalar_mul(
                                xg[:], xp[nm][cc][:, lo:lo + TC],
                                gs[:, 2 * ig + cc:2 * ig + cc + 1])
                            m1 = pool.tile([128, TC], BF16, tag="rm1",
                                           name=f"rm1")
                            nc.vector.tensor_tensor(
                                out=m1[:], in0=xg[:], in1=cos2[:, lo:lo + TC],
                                op=ALU.mult)
                            cell.update(xg=xg, m1=m1)

                        def s2(t=t, cell=cell):
                            sw = psum_pool.tile([128, TC], F32, tag=psum_tag,
                                                name=f"rsw")
                            nc.tensor.matmul(sw[:], swp[:], cell["xg"][:],
                                             start=True, stop=True)
                            cell["sw"] = sw

                        def s3(t=t, cell=cell):
                            lo = t * TC
                            m2 = pool.tile([128, TC], BF16, tag="rm2",
                                           name=f"rm2")
                            nc.vector.tensor_tensor(
                                out=m2[:], in0=cell["sw"][:],
                                in1=sin2[:, lo:lo + TC], op=ALU.mult)
                            nc.vector.tensor_tensor(
                                out=out_tile[:, lo:lo + TC],
                                in0=cell["m1"][:], in1=m2[:], op=ALU.add)

                        steps += [s1, s2, s3]
                    return steps

                def rope_now(nm, cc, pool, psum_pool, psum_tag="swp"):
                    for s in rope_steps(nm, cc, pool, psum_pool, psum_tag):
                        s()
                rope_steps_fn[0] = rope_steps

                # --- q projection: warmup + kc-outer into 8 banks ---
                with tc.tile_pool(name="pps8", bufs=1, space="PSUM") as pps8:
                    ps8 = {cc: [pps8.tile([128, TC], F32, tag=f"p{cc}{t}",
                                          name=f"p{cc}{t}") for t in range(NT)]
                           for cc in range(2)}
                    # PE clock warmup: dummy matmuls keep the p-state ramp
                    # from oscillating during the x-paced projection
                    nc.vector.memset(dummy[:], 0.0)
                    for _ in range(10):
                        nc.tensor.matmul(ps8[0][0][:], dummy[:, 0:128],
                                         dummy[:], start=True, stop=True)
                    wt = wts["q"]
                    for kc in range(NK):
                        for cc in range(2):
                            wsl = wt[:, kc * 256 + cc * 128:
                                     kc * 256 + (cc + 1) * 128]
                            for t in range(NT):
                                nc.tensor.matmul(
                                    ps8[cc][t][:], wsl,
                                    xk[kc][:, t * TC:(t + 1) * TC],
                                    start=(kc == 0), stop=(kc == NK - 1))
                    bias_evac("q", 0, ps8[0])
                    bias_evac("q", 1, ps8[1])

                # --- k projection + sumsqs + rope bases ---
                with tc.tile_pool(name="pps4", bufs=1, space="PSUM") as pps4, \
                     tc.tile_pool(name="sq_ps", bufs=1, space="PSUM") as sps, \
                     tc.tile_pool(name="rope_ps", bufs=2, space="PSUM") as rps:
                    # q sumsq + c1 (AllGather) first: the collective flies
                    # while PE does the k projection
                    sumsq("q", sps)
                    nc.gpsimd.dma_start(cin_q[:], sq_sb["q"][:])
                    nc.gpsimd.collective_compute(
                        "AllGather", ALU.bypass, replica_groups=groups_all,
                        ins=[cin_q.opt()], outs=[cout_q.opt()])

                    def kproj(cc):
                        psk = [pps4.tile([128, TC], F32, tag=f"proj{t}",
                                         name=f"proj{t}") for t in range(NT)]
                        wt = wts["k"]
                        for kc in range(NK):
                            wsl = wt[:, kc * 256 + cc * 128:
                                     kc * 256 + (cc + 1) * 128]
                            for t in range(NT):
                                nc.tensor.matmul(
                                    psk[t][:], wsl,
                                    xk[kc][:, t * TC:(t + 1) * TC],
                                    start=(kc == 0), stop=(kc == NK - 1))
                        bias_evac("k", cc, psk)

                    kproj(0)
                    rope_now("q", 1, rp, rps)   # DVE + a few swp matmuls
                    kproj(1)
                    rope_now("q", 0, rp, rps)
                    sumsq("k", sps)
                    nc.gpsimd.dma_start(cin_k[0:1, 0:T], sq_sb["k"][:])
                    nc.gpsimd.collective_compute(
                        "AllReduce", ALU.add, replica_groups=groups_all,
                        ins=[cin_k.opt()], outs=[cout_k.opt()])

                # --- q norm chain + qmult for slot1's q ---
                with tc.tile_pool(name="qs_ps", bufs=1, space="PSUM") as qsp, \
                     tc.tile_pool(name="bps_ps", bufs=1, space="PSUM") as bpp:
                    gath = rp.tile([8, T], F32R, tag="gath", bufs=1)
                    nc.gpsimd.dma_start(gath[:], cout_q[:])
                    srt = rp.tile([1, T], F32, tag="srt", bufs=1)
                    for t in range(NT):
                        qsum = qsp.tile([1, TC], F32, tag="qsum")
                        nc.tensor.matmul(qsum[:], ones_r[0:8, :],
                                         gath[:, t * TC:(t + 1) * TC],
                                         start=True, stop=True)
                        nc.scalar.activation(srt[:, t * TC:(t + 1) * TC],
                                             qsum[:], AF.Sqrt, bias=eps_cD[:],
                                             scale=float(D) / XD)
                    with nc.allow_low_precision(reason="f32r is f32 bits"):
                        nc.vector.reciprocal(rec2[:], srt[:])
                    bps = [bpp.tile([128, TC], F32, tag=f"bps{t}",
                                    name=f"bps{t}") for t in range(NT)]
                    for t in range(NT):
                        nc.tensor.matmul(bps[t][:], onesrow[:],
                                         rec2[:, t * TC:(t + 1) * TC],
                                         start=True, stop=True)
                    for t in range(NT):
                        nc.vector.tensor_tensor(
                            out=qkf["q"][1][:, t * TC:(t + 1) * TC],
                            in0=qkf["q"][1][:, t * TC:(t + 1) * TC],
                            in1=bps[t][:], op=ALU.mult)

        # bulky late-phase tiles (SBUF reused from the projection pools)
        with tc.tile_pool(name="late", bufs=1) as late:
            out1_sb = late.tile([128, T], F32, tag="out1sb")
            out0_sb = late.tile([128, T], F32, tag="out0sb")
            of1_sb = late.tile([128, T], BF16, tag="of1sb")
            of0_sb = late.tile([128, T], BF16, tag="of0sb")
            den_sb = [late.tile([1, T], F32, tag=f"den{s}", name=f"den{s}")
                      for s in range(2)]
            woT_sb = late.tile([128, 2 * XD], BF16, tag="woT")
            gath_d = late.tile([2, T], F32R, tag="gathd")

            # ================= P4-P7: attention =================
            with tc.tile_pool(name="outps", bufs=1, space="PSUM") as ops, \
                 tc.tile_pool(name="dps", bufs=1, space="PSUM") as dpool, \
                 tc.tile_pool(name="aux_ps", bufs=1, space="PSUM") as auxp, \
                 tc.tile_pool(name="stp", bufs=4, space="PSUM") as stp, \
                 tc.tile_pool(name="attk", bufs=3) as ap_, \
                 tc.tile_pool(name="expp", bufs=14) as ep_, \
                 tc.tile_pool(name="s2pool", bufs=7) as s2p, \
                 tc.tile_pool(name="s4pool", bufs=5) as s4p, \
                 tc.tile_pool(name="ropedrip", bufs=2) as rpd:

                # ---- background work queue (dripped into attention) ----
                bg = []

                def bg_qmult0():
                    # slot0's q norm multiply via aux-bank broadcast
                    for t in range(NT):
                        def step(t=t):
                            a = auxp.tile([128, TC], F32, tag="aux",
                                          name=f"qm{t}")
                            nc.tensor.matmul(a[:], onesrow[:],
                                             rec2[:, t * TC:(t + 1) * TC],
                                             start=True, stop=True)
                            nc.vector.tensor_tensor(
                                out=qkf["q"][0][:, t * TC:(t + 1) * TC],
                                in0=qkf["q"][0][:, t * TC:(t + 1) * TC],
                                in1=a[:], op=ALU.mult)
                        bg.append(step)

                def bg_vproj(cc):
                    for t in range(NT):
                        cell = {}
                        for kc in range(NK):
                            def step(cc=cc, t=t, kc=kc, cell=cell):
                                if kc == 0:
                                    cell["ps"] = auxp.tile(
                                        [128, TC], F32, tag="aux",
                                        name=f"vps{cc}{t}")
                                wsl = wv_t[:, kc * 256 + cc * 128:
                                           kc * 256 + (cc + 1) * 128]
                                nc.tensor.matmul(
                                    cell["ps"][:], wsl,
                                    xk[kc][:, t * TC:(t + 1) * TC],
                                    start=(kc == 0), stop=(kc == NK - 1))
                            bg.append(step)

                        def bstep(cc=cc, t=t, cell=cell):
                            nc.vector.tensor_scalar_add(
                                xp_v[cc][:, t * TC:(t + 1) * TC],
                                cell["ps"][:], biases[:, 4 + cc:5 + cc])
                        bg.append(bstep)

                    def tstep(cc=cc):
                        nc.sync.dma_start_transpose(
                            v_nat[cc][:].rearrange("p (j d) -> p j d",
                                                   j=n_new),
                            xp_v[cc][:])
                    bg.append(tstep)

                bg.extend(rope_steps_fn[0]("k", 1, rpd, auxp, "aux"))
                bg_qmult0()
                bg_vproj(1)
                bg.extend(rope_steps_fn[0]("k", 0, rpd, auxp, "aux"))
                bg_vproj(0)
                ucount = [0]

                def lamk_compute():
                    """Consume c2: transposed load + rsqrt -> lamk."""
                    nc.gpsimd.dma_start(
                        lamk_sq[:].rearrange("p (c o) -> p c o", o=1),
                        cout_k[0:1, :].rearrange("o (c p) -> p c o", p=128))
                    nc.scalar.activation(lamk_srt[:], lamk_sq[:], AF.Sqrt,
                                         bias=eps_col[:], scale=1.0 / XD)
                    nc.vector.reciprocal(lamk[:], lamk_srt[:])

                def run_phase(slot, segments, pass_end, hooks={},
                              no_pair_until=0):
                    """One attention phase over `segments`, NPASS query-half
                    passes.  pass_end(pidx, ts, out_tiles, den_ps) emitted
                    per pass; hooks {(pass, chunk): fn}."""
                    chunks = []
                    base_supers = []
                    for seg in segments:
                        if seg[0] == "dram":
                            _, ksrc, vsrc, L, btile = seg
                            for soff, ssz in subchunks(L, SUPER):
                                sidx = len(base_supers)
                                base_supers.append((ksrc, vsrc, soff, ssz))
                                for j, (o2, c2) in enumerate(subchunks(ssz)):
                                    chunks.append(dict(
                                        kind="dram", ck=c2, btile=btile,
                                        bidx=(soff + o2) // 128, scale=1.0,
                                        sidx=sidx, sj=j, so=o2))
                        else:
                            _, ktile, L, btile, scale_t = seg
                            for j, (off, ck) in enumerate(subchunks(L)):
                                chunks.append(dict(
                                    kind="sbuf", ck=ck, btile=btile, bidx=j,
                                    scale=scale_t, ktile=ktile, koff=off,
                                    sj=j))
                    nch = len(chunks)
                    nsup = len(base_supers)
                    # pair plan + den count (pairs merge into quads at
                    # emission time; count dens by simulating the grouping)
                    for c in chunks:
                        c["pair1"] = c["pair2"] = False
                    i = no_pair_until
                    while i < nch - 1:
                        a, b = chunks[i], chunks[i + 1]
                        if a["ck"] == 128 and b["ck"] == 128 and not a["pair2"]:
                            a["pair1"] = True
                            b["pair2"] = True
                            i += 2
                        else:
                            i += 1
                    n_den = 0
                    held = False
                    for c in chunks:
                        if c["pair2"]:
                            if held:
                                n_den += 1
                                held = False
                            else:
                                held = True
                        elif not c["pair1"]:
                            n_den += 1
                    if held:
                        n_den += 1

                    flat = []
                    for pidx in range(NPASS):
                        for c in chunks:
                            c2 = dict(c)
                            if c2["kind"] == "dram":
                                c2["sidx"] = c2["sidx"] + pidx * nsup
                            flat.append(c2)
                    supers = [base_supers[i % nsup]
                              for i in range(nsup * NPASS)] if nsup else []
                    ntot = len(flat)

                    super_state = dict(pre.get(slot, {})) if nsup else {}
                    issued = [len(super_state)]
                    st_tiles = {}
                    ex_tiles = {}

                    def issue_super(sidx):
                        while issued[0] <= min(sidx + 1, len(supers) - 1):
                            s = issued[0]
                            if s not in super_state:
                                ksrc, vsrc, soff, ssz = supers[s]
                                ks = ap_.tile([128, SUPER], BF16, tag="ks")
                                nc.sync.dma_start(
                                    ks[:, 0:ssz],
                                    ksrc.ap()[:, soff:soff + ssz])
                                vs = ap_.tile([128, SUPER], BF16, tag="vs")
                                vs_dma(vs, vsrc, soff, ssz)
                                super_state[s] = (ks, vs)
                            issued[0] += 1

                    def look_ahead(ci):
                        for cj in range(ci, min(ci + 5, ntot)):
                            if flat[cj]["kind"] == "dram":
                                issue_super(flat[cj]["sidx"])
                                return

                    def k_ap(c):
                        if c["kind"] == "dram":
                            ks, _ = super_state[c["sidx"]]
                            return ks[:, c["so"]:c["so"] + c["ck"]]
                        return c["ktile"][:, c["koff"]:c["koff"] + c["ck"]]

                    def v_ap(c):
                        if c["kind"] == "dram":
                            _, vs = super_state[c["sidx"]]
                            return vs[0:c["ck"],
                                      c["sj"] * 128:(c["sj"] + 1) * 128]
                        return v_nat[slot][0:c["ck"],
                                           c["sj"] * 128:(c["sj"] + 1) * 128]

                    def emit_st(ci, t):
                        c = flat[ci]
                        look_ahead(ci)
                        st = stp.tile([128, TC], F32, tag="st")
                        nc.tensor.matmul(
                            st[0:c["ck"], :], k_ap(c),
                            qkf["q"][slot][:, t * TC:(t + 1) * TC],
                            start=True, stop=True)
                        st_tiles[(ci, t)] = st

                    for pidx in range(NPASS):
                        ts = list(range(pidx * TP, (pidx + 1) * TP))
                        den_ps = dpool.tile([128, TC], F32, tag="den",
                                            name=f"dn{slot}{pidx}")
                        out_tiles = [ops.tile([128, TC], F32, tag=f"o_{i}",
                                              name=f"o{slot}{pidx}{i}")
                                     for i in range(TP)]
                        den_idx = {t: 0 for t in ts}
                        grp = {t: None for t in ts}
                        pending = []

                        def flush_den(n, den_ps=den_ps, den_idx=den_idx,
                                      pending=pending):
                            for _ in range(min(n, len(pending))):
                                ap, ck, tt = pending.pop(0)
                                row = 32 * (tt % TP)
                                nc.tensor.matmul(
                                    den_ps[row:row + 1, :],
                                    ones_bf[0:ck, :], ap,
                                    start=(den_idx[tt] == 0),
                                    stop=(den_idx[tt] == n_den - 1),
                                    skip_group_check=True)
                                den_idx[tt] += 1

                        base = pidx * nch
                        emit_st(base, ts[0])
                        for cl in range(nch):
                            ci = base + cl
                            c = flat[ci]
                            if (pidx, cl) in hooks:
                                hooks[(pidx, cl)]()
                            ck = c["ck"]
                            for it, t in enumerate(ts):
                                st = st_tiles.pop((ci, t))
                                ex = ep_.tile([128, TC], BF16, tag="ex")
                                bias = 0.0 if c["btile"] is None else \
                                    c["btile"][0:ck, c["bidx"]:c["bidx"] + 1]
                                scale = c["scale"]
                                if not isinstance(scale, float):
                                    scale = scale[0:ck,
                                                  c["bidx"]:c["bidx"] + 1]
                                nc.scalar.activation(
                                    ex[0:ck, :], st[0:ck, :], AF.Exp,
                                    bias=bias, scale=scale)
                                # one-ahead score matmul
                                if it + 1 < TP:
                                    emit_st(ci, ts[it + 1])
                                elif cl + 1 < nch:
                                    emit_st(ci + 1, ts[0])
                                # background drip (1 step / 2 units)
                                ucount[0] += 1
                                if bg and ucount[0] % 2 == 0:
                                    bg.pop(0)()
                                if len(pending) > 3:
                                    flush_den(1)
                                nc.tensor.matmul(
                                    out_tiles[it][:], v_ap(c), ex[0:ck, :],
                                    start=(cl == 0), stop=(cl == nch - 1),
                                    skip_group_check=True)
                                if c["pair2"]:
                                    s2 = s2p.tile([128, TC], BF16, tag="s2")
                                    nc.vector.tensor_tensor(
                                        out=s2[:],
                                        in0=ex_tiles[(ci - 1, t)][:, :],
                                        in1=ex[:, :], op=ALU.add)
                                    if grp[t] is not None:
                                        s4 = s4p.tile([128, TC], BF16,
                                                      tag="s4")
                                        nc.vector.tensor_tensor(
                                            out=s4[:], in0=grp[t][:, :],
                                            in1=s2[:, :], op=ALU.add)
                                        pending.append((s4[:, :], 128, t))
                                        grp[t] = None
                                    else:
                                        grp[t] = s2
                                elif not c["pair1"]:
                                    pending.append((ex[0:ck, :], ck, t))
                                if c["pair1"]:
                                    ex_tiles[(ci, t)] = ex
                            if cl >= 1:
                                for t in ts:
                                    ex_tiles.pop((ci - 1, t), None)
                        for t in ts:
                            if grp[t] is not None:
                                pending.append((grp[t][:, :], 128, t))
                                grp[t] = None
                        flush_den(len(pending))
                        pass_end(pidx, ts, out_tiles, den_ps)

                def copy_merge(osb, dsb):
                    def fn(pidx, ts, outs, den_ps):
                        for it, t in enumerate(ts):
                            lo = t * TC
                            nc.vector.tensor_copy(osb[:, lo:lo + TC],
                                                  outs[it][:])
                            row = 32 * it
                            nc.vector.tensor_copy(
                                dsb[0:1, lo:lo + TC],
                                den_ps[row:row + 1, :])
                    return fn

                def add_merge(osb, dsb, extra=None):
                    def fn(pidx, ts, outs, den_ps):
                        for it, t in enumerate(ts):
                            lo = t * TC
                            nc.vector.tensor_tensor(
                                out=osb[:, lo:lo + TC], in0=osb[:, lo:lo + TC],
                                in1=outs[it][:], op=ALU.add)
                            row = 32 * it
                            nc.vector.tensor_tensor(
                                out=dsb[0:1, lo:lo + TC],
                                in0=dsb[0:1, lo:lo + TC],
                                in1=den_ps[row:row + 1, :], op=ALU.add)
                        if extra is not None:
                            extra(pidx, ts)
                    return fn

                # ---- phase A: slot1 over the old-window cache ----
                run_phase(1, [("dram", kc1_d, vc1_d, L1, bias1)],
                          copy_merge(out1_sb, den_sb[1]))

                # ---- phase C: slot1 over the new keys (early, so the
                # pair-reduce and slot1 normalize hide under phase B) ----
                run_phase(1, [("sbuf", qkf["k"][1], NEW, bias2, lamk)],
                          add_merge(out1_sb, den_sb[1]),
                          hooks={(0, 0): lamk_compute})
                nc.gpsimd.dma_start(cin_d[:], den_sb[1][:])
                nc.gpsimd.collective_compute(
                    "AllGather", ALU.bypass, replica_groups=groups_pair,
                    ins=[cin_d.opt()], outs=[cout_d.opt()])

                # ---- phase B: slot0 over the old-window cache ----
                def woT_hook():
                    nc.sync.dma_start(woT_sb[:, 0:XD], woT_d.ap()[0:128, :])
                    nc.sync.dma_start(woT_sb[:, XD:2 * XD],
                                      woT_d.ap()[128:256, :])

                def of1_hook():
                    # c3 arrived: sum the pair-gathered denominators and
                    # normalize slot1 (DVE/Pool work under the Act stream)
                    nc.gpsimd.dma_start(gath_d[:], cout_d[:])
                    d1s = late.tile([1, T], F32, tag="rcx", name="d1s", bufs=2)
                    for t in range(NT):
                        a = auxp.tile([128, TC], F32, tag="aux",
                                      name=f"c3s{t}")
                        nc.tensor.matmul(a[0:1, :], ones_r[0:2, :],
                                         gath_d[:, t * TC:(t + 1) * TC],
                                         start=True, stop=True)
                        nc.vector.tensor_copy(
                            d1s[0:1, t * TC:(t + 1) * TC], a[0:1, :])
                    rc1 = late.tile([1, T], F32, tag="rcx", name="rc1", bufs=2)
                    nc.vector.reciprocal(rc1[:], d1s[:])
                    for t in range(NT):
                        rb = late.tile([128, TC], F32, tag="rbt", bufs=2)
                        nc.gpsimd.partition_broadcast(
                            rb[:], rc1[0:1, t * TC:(t + 1) * TC])
                        nc.vector.tensor_tensor(
                            out=of1_sb[:, t * TC:(t + 1) * TC],
                            in0=out1_sb[:, t * TC:(t + 1) * TC],
                            in1=rb[:], op=ALU.mult)

                run_phase(0, [("dram", kc0_d, vc0_d, L0, None)],
                          copy_merge(out0_sb, den_sb[0]),
                          hooks={(0, 1): woT_hook, (1, 20): of1_hook})

                # ---- phase D: slot0 over the new keys ----
                rcx = {}

                def of0_extra(pidx, ts):
                    rc0 = rcx.setdefault(
                        "rc0", late.tile([1, T], F32, tag="rc0", name="rc0",
                                         bufs=1))
                    lo, hi = ts[0] * TC, (ts[-1] + 1) * TC
                    nc.vector.reciprocal(rc0[:, lo:hi], den_sb[0][:, lo:hi])
                    for t in ts:
                        rb = late.tile([128, TC], F32, tag="rbt", bufs=2)
                        nc.gpsimd.partition_broadcast(
                            rb[:], rc0[0:1, t * TC:(t + 1) * TC])
                        nc.vector.tensor_tensor(
                            out=of0_sb[:, t * TC:(t + 1) * TC],
                            in0=out0_sb[:, t * TC:(t + 1) * TC],
                            in1=rb[:], op=ALU.mult)

                run_phase(0, [("sbuf", qkf["k"][0], NEW, None, lamk)],
                          add_merge(out0_sb, den_sb[0], of0_extra))

            # ================= P8: out projection ==========
            # two query-half sweeps: the first half's matmuls interleave
            # with attention phase D's second pass (its of inputs are ready)
            with tc.tile_pool(name="fin", bufs=3) as fp_, \
                 tc.tile_pool(name="yps", bufs=3, space="PSUM") as yps:
                for ph in range(1):
                    tl = list(range(NT))
                    for jc in range(NJ):
                        ysb = fp_.tile([128, NT * TC], BF16, tag="ysb")
                        for it, t in enumerate(tl):
                            yp = yps.tile([128, TC], F32, tag="yp")
                            nc.tensor.matmul(
                                yp[:], woT_sb[:, jc * 128:(jc + 1) * 128],
                                of0_sb[:, t * TC:(t + 1) * TC],
                                start=True, stop=False)
                            nc.tensor.matmul(
                                yp[:],
                                woT_sb[:, XD + jc * 128:XD + (jc + 1) * 128],
                                of1_sb[:, t * TC:(t + 1) * TC],
                                start=False, stop=True)
                            if (jc + it) % 2 == 0:
                                nc.vector.tensor_scalar_add(
                                    ysb[:, it * TC:(it + 1) * TC], yp[:],
                                    bo8[:, jc:jc + 1])
                            else:
                                nc.scalar.activation(
                                    ysb[:, it * TC:(it + 1) * TC], yp[:],
                                    AF.Identity, bias=bo8[:, jc:jc + 1])
                        eng = nc.sync if jc % 2 == 0 else nc.scalar
                        eng.dma_start(
                            yT_d.ap()[jc * 128:(jc + 1) * 128,
                                      tl[0] * TC:(tl[-1] + 1) * TC], ysb[:])

    nc.compile()
    return nc


# ---------------- host side ----------------

def host_prepare(cfg, x, freqs_cos, freqs_sin, wq, bq, wk, bk, wv, bv,
                 wo, bo, gq, gk, win_old_k, win_old_v, n_cores=8):
    """win_old_k/v: [L0, XD] assembled old window (eviction applied)."""
    import ml_dtypes
    T, XD = cfg["T"], cfg["XD"]
    L0, L1 = cfg["L0"], cfg["L1"]
    NEW = T
    split = L1 - NEW
    assert 0 <= split <= L0 and L0 - split <= L1, (L0, L1, split)
    n_sub1 = len(subchunks(L1))
    n_new = len(subchunks(NEW))
    NK = XD // 128

    f32 = np.float32
    bf16 = ml_dtypes.bfloat16
    xT = np.ascontiguousarray(x.reshape(T, XD).T.astype(bf16))
    cos2 = np.concatenate([freqs_cos.T, freqs_cos.T], 0).astype(f32)
    sin2 = np.concatenate([freqs_sin.T, freqs_sin.T], 0).astype(f32)
    cossin = np.ascontiguousarray(
        np.concatenate([cos2, sin2], 1).astype(bf16))
    swpc = np.zeros((128, 128), f32)
    swpc[np.arange(64), np.arange(64) + 64] = 1.0
    swpc[np.arange(64) + 64, np.arange(64)] = -1.0
    swpc = np.ascontiguousarray(swpc.astype(bf16))

    def warr(w, cols):
        ws = w[cols, :].T.astype(bf16)          # [XD, 256]
        return np.ascontiguousarray(
            ws.reshape(NK, 128, 256).transpose(1, 0, 2).reshape(128, NK * 256))

    in_maps = []
    for c in range(n_cores):
        p, role = divmod(c, 2)
        h0 = 3 * p + (0 if role == 0 else 2)
        h1 = 3 * p + 1
        cols = np.r_[h0 * 128:(h0 + 1) * 128, h1 * 128:(h1 + 1) * 128]
        cols0 = np.r_[h0 * 128:(h0 + 1) * 128]
        cols1 = np.r_[h1 * 128:(h1 + 1) * 128]

        m = {"xT": xT, "cossin": cossin, "swpc": swpc}
        m["wq"] = warr(wq, cols)
        m["wk"] = warr(wk, cols)
        m["wv"] = warr(wv, cols)
        m["woT"] = np.ascontiguousarray(wo[:, cols].T.astype(bf16))

        sqmask = np.zeros((256,), f32)
        sqmask[0:128] = 1.0
        q4 = 32
        if role == 0:
            sqmask[128:128 + q4] = 1.0
            sqmask[128 + 2 * q4:128 + 3 * q4] = 1.0
        else:
            sqmask[128 + q4:128 + 2 * q4] = 1.0
            sqmask[128 + 3 * q4:] = 1.0

        if role == 0:
            valid1 = split
        else:
            valid1 = L0 - split
        bias1 = np.zeros((128, n_sub1), f32)
        for j, (off, ck) in enumerate(subchunks(L1)):
            lv = int(np.clip(valid1 - off, 0, 128))
            bias1[lv:, j] = NEG_BIAS
        bias2 = np.zeros((128, n_new), f32)
        if role == 1:
            bias2[:] = NEG_BIAS

        NJ = NK
        NS = 6 + 4 + NJ + 2 + n_sub1 + n_new
        smalls = np.zeros((128, NS), f32)
        for i, b in enumerate((bq, bk, bv)):
            smalls[:, 2 * i] = b[cols][0:128]
            smalls[:, 2 * i + 1] = b[cols][128:256]
        for i, g in enumerate((gq, gk)):
            smalls[:, 6 + 2 * i] = g[cols][0:128]
            smalls[:, 7 + 2 * i] = g[cols][128:256]
        smalls[:, 10:10 + NJ] = bo.reshape(NJ, 128).T / n_cores
        smalls[:, 10 + NJ] = sqmask[0:128]
        smalls[:, 11 + NJ] = sqmask[128:256]
        smalls[:, 12 + NJ:12 + NJ + n_sub1] = bias1
        smalls[:, 12 + NJ + n_sub1:NS] = bias2
        m["smalls"] = smalls

        m["kc0T"] = np.ascontiguousarray(win_old_k[:, cols0].T.astype(bf16))
        m["vc0"] = np.ascontiguousarray(win_old_v[:, cols0].astype(bf16))

        k1 = np.zeros((L1, 128), f32)
        v1 = np.zeros((L1, 128), f32)
        if role == 0:
            k1[0:split] = win_old_k[0:split][:, cols1]
            v1[0:split] = win_old_v[0:split][:, cols1]
        else:
            k1[0:valid1] = win_old_k[split:L0][:, cols1]
            v1[0:valid1] = win_old_v[split:L0][:, cols1]
        m["kc1T"] = np.ascontiguousarray(k1.T.astype(bf16))
        m["vc1"] = np.ascontiguousarray(v1.astype(bf16))
        in_maps.append(m)
    return in_maps


def host_finalize(cfg, results):
    y = None
    for r in results:
        y = r["yT"].astype(np.float64) if y is None \
            else y + r["yT"].astype(np.float64)
    return np.ascontiguousarray(y.T)[None].astype(np.float32)  # [1, T, XD]


def numpy_reference(cfg, x, freqs_cos, freqs_sin, wq, bq, wk, bk, wv, bv,
                    wo, bo, gq, gk, win_old_k, win_old_v):
    """Reference for arbitrary cfg: attention over [old window; new]."""
    T, XD, D = cfg["T"], cfg["XD"], cfg["D"]
    H = XD // D
    x2 = x.reshape(T, XD).astype(np.float64)

    def rms(t, g):
        return t / np.sqrt((t ** 2).mean(-1, keepdims=True) + EPS) * g

    q = rms(x2 @ wq.T + bq, gq)
    k = rms(x2 @ wk.T + bk, gk)
    v = x2 @ wv.T + bv

    def rope(t):
        th = t.reshape(T, H, D)
        t1, t2 = th[..., :D // 2], th[..., D // 2:]
        c = freqs_cos[:, None, :]
        s = freqs_sin[:, None, :]
        return np.concatenate([t1 * c - t2 * s, t1 * s + t2 * c],
                              -1).reshape(T, XD)

    rq, rk = rope(q), rope(k)
    kw = np.concatenate([win_old_k, rk], 0).reshape(-1, H, D)
    vw = np.concatenate([win_old_v, v], 0).reshape(-1, H, D)
    qh = rq.reshape(T, H, D)
    scores = np.einsum("thd,shd->hts", qh, kw) / math.sqrt(D)
    e = np.exp(scores - scores.max(-1, keepdims=True))
    probs = e / e.sum(-1, keepdims=True)
    out = np.einsum("hts,shd->thd", probs, vw).reshape(T, XD)
    return (out @ wo.T + bo)[None].astype(np.float32)


# =====================================================================
# kernel() entry point — full inputs in, full output out.
# =====================================================================

import os as _os
import time as _time
from concourse import bass_utils as _bass_utils

_DIM = 1536
_HEADS = 12
_HD = 128
_FRAME = 1560
_LOCAL_ATTN_SIZE = 6
_SINK_SIZE = 1
_CACHE = _LOCAL_ATTN_SIZE * _FRAME
_SINK = _SINK_SIZE * _FRAME
_MAX_ATTN = _LOCAL_ATTN_SIZE * _FRAME
_GLOBAL_END = _CACHE
_LOCAL_END = _CACHE

_prog_cache = {}
last_exec_ns = None
last_wall_ns = None


def _window_index(current_start, T):
    """Mirrors the reference's rolling-cache index math; returns original
    cache row indices of the attention window's old part."""
    cur_end = current_start + T
    if cur_end > _GLOBAL_END and T + _LOCAL_END > _CACHE:
        evict = T + _LOCAL_END - _CACHE
        rolled = _LOCAL_END - evict - _SINK
        le = _LOCAL_END + cur_end - _GLOBAL_END - evict
    else:
        evict, rolled = 0, 0
        le = _LOCAL_END + cur_end - _GLOBAL_END
    ls = le - T
    ws = max(0, le - _MAX_ATTN)
    idx = np.arange(ws, ls)
    if evict:
        shift = (idx >= _SINK) & (idx < _SINK + rolled)
        idx = np.where(shift, idx + evict, idx)
    return idx


def kernel(**inputs):
    global last_exec_ns, last_wall_ns
    x = np.ascontiguousarray(np.asarray(inputs["x"], np.float32))
    B, T, XD = x.shape
    assert B == 1 and XD == _DIM
    cs = int(np.asarray(inputs["current_start"]))
    idx_old = _window_index(cs, T)
    L0 = len(idx_old)
    L1 = max(T, (L0 + T + 1) // 2)
    NT = 1
    for cand in (4, 3, 2):
        if T % cand == 0 and T // cand <= 512:
            NT = cand
            break
    if T <= 512:
        NT = 1
    cfg = dict(T=T, NT=NT, XD=XD, D=_HD, L0=L0, L1=L1, SUPER=512)

    key = tuple(sorted(cfg.items()))
    if key not in _prog_cache:
        _prog_cache[key] = build_program(cfg, n_cores=8)
    nc = _prog_cache[key]

    f32 = np.float32
    ck = np.asarray(inputs["cache_k"], f32)[0]
    cv = np.asarray(inputs["cache_v"], f32)[0]
    win_k = np.ascontiguousarray(ck[idx_old])
    win_v = np.ascontiguousarray(cv[idx_old])
    args = (x, np.asarray(inputs["freqs_cos"], f32),
            np.asarray(inputs["freqs_sin"], f32),
            np.asarray(inputs["wq"], f32), np.asarray(inputs["bq"], f32),
            np.asarray(inputs["wk"], f32), np.asarray(inputs["bk"], f32),
            np.asarray(inputs["wv"], f32), np.asarray(inputs["bv"], f32),
            np.asarray(inputs["wo"], f32), np.asarray(inputs["bo"], f32),
            np.asarray(inputs["gq"], f32), np.asarray(inputs["gk"], f32),
            win_k, win_v)
    in_maps = host_prepare(cfg, *args, n_cores=8)

    trace = bool(int(_os.environ.get("WAN_KERNEL_TRACE", "0")))
    t0 = _time.time()
    try:
        res = _bass_utils.run_bass_kernel_spmd(
            nc, in_maps, core_ids=list(range(8)), trace=trace)
    except (ImportError, ModuleNotFoundError):
        # NTFF profile hook unavailable on this client image
        res = _bass_utils.run_bass_kernel_spmd(
            nc, in_maps, core_ids=list(range(8)))
    last_wall_ns = int((_time.time() - t0) * 1e9)
    last_exec_ns = res.exec_time_ns
    return host_finalize(cfg, [res.results[c] for c in range(8)])
